# revision 21
# baseline (speedup 1.0000x reference)
"""Soft Needleman-Wunsch forward (logsumexp semiring) on Trainium2, 8 cores.

Exp-space linearization: W[i,j] = exp(V[i,j] - alpha*(i+j)) satisfies
    W[i,j] = (z[j] + W[i,j-1]) * c[i,j]
    z[j]   = W[i-1,j] + r[i,j] * W[i-1,j-1]
    c = exp(theta + A - alpha),  r = exp(-A - alpha)
so each DP row is one tensor_tensor_scan (op0=add, op1=mult) plus two
elementwise DVE ops (m = r*Wshift, z = m + Wprev).

Per core (16 batch): 8 column strips of 64, partition p = 16*s + b,
strip s lags s-1 by LAG rows (wavefront); row i of strip s runs at
step t = i + LAG*s.  Strip handoff (col64 -> next strip col0) is one
SBUF->SBUF DMA per GW steps.

vs the previous version: c/r/W and the DP ops are bf16 (DVE 2x mode on
the tensor_tensor ops; scan rate unchanged), LAG=16 (624 steps), input
ring deepened to 128 steps with G=64 windows and theta/A DMAs split
across the SP and ACT queues so DMA issue does not serialize behind
ring-reuse waits on one sequencer.
"""
import numpy as np

# ---- constants (self-contained; hardcoded for the 128x512x512 problem) ----
B_FULL, N, M = 128, 512, 512
NCORES = 8
B = B_FULL // NCORES      # 16 batch elements per core
S = 8                     # column strips
WJ = M // S               # 64 = strip width
LAG = 16                  # inter-strip row lag (steps)
GW = 8                    # handoff batching window (steps); must be < LAG
T_TOTAL = N + (S - 1) * LAG   # 624 steps
WRING = 64                # W row ring depth (slots); multiple of GW and LAG
WPITCH = WJ + 1           # 65 elems per slot per partition
IRING = 128               # input ring depth (steps)
G = 64                    # input DMA window (steps); IRING/2 (aligned)
GP = 16                   # prep (sum/exp) grouping (steps)
PF_IN = 64                # input DMA issue lead (steps before window start)
PF_PREP = 32              # exp/sum prep prefetch distance
PF_GUARD = 8              # handoff DMA prefetch distance
GB = 32                   # strip-0 boundary (btab) window (steps)
ALPHA = 1.52              # calibrated exp-space drift rate

_CACHE = {}


def _bf16(x):
    import ml_dtypes
    return np.asarray(x, dtype=ml_dtypes.bfloat16)


def _active_range(t):
    """Strips s with 1 <= t - LAG*s <= N, as [lo, hi) (contiguous range)."""
    lo = max(0, -(-(t - N) // LAG))      # ceil((t - N) / LAG)
    hi = min(S - 1, (t - 1) // LAG) + 1
    return (lo, hi) if hi > lo else (0, 0)


def _build_nc():
    import concourse.bass as bass
    import concourse.bacc as bacc
    import concourse.mybir as mybir
    import concourse.tile as tile
    from contextlib import ExitStack

    f32 = mybir.dt.float32
    bf16 = mybir.dt.bfloat16
    Alu = mybir.AluOpType
    ActFn = mybir.ActivationFunctionType
    al = float(ALPHA)

    nc = bacc.Bacc("TRN2", target_bir_lowering=False)
    th_d = nc.dram_tensor("theta", [B, N, M], f32, kind="ExternalInput")
    a_d = nc.dram_tensor("A", [B, N, M], f32, kind="ExternalInput")
    out_d = nc.dram_tensor("out", [B, 1], f32, kind="ExternalOutput")

    # baked constant tables (bf16, matching the ring dtypes)
    sv = (S - 1 - np.arange(128) // B).astype(np.float64)
    fv = np.arange(WPITCH, dtype=np.float64)
    w0_np = np.exp(-al * (WJ * sv[:, None] + fv[None, :]))
    w0_c = nc.inline_tensor(_bf16(w0_np), name="w0tab")     # [128, 65]
    bt_np = np.tile(np.exp(-al * np.arange(1, N + 1, dtype=np.float64)
                           )[None, :], (B, 1))
    bt_c = nc.inline_tensor(_bf16(bt_np), name="btab")      # [16, 512]

    # persistent SBUF rings
    w_ring = nc.alloc_sbuf_tensor("w_ring", [128, WPITCH * WRING], bf16)
    th_ring = nc.alloc_sbuf_tensor("th_ring", [128, WJ * IRING], f32)
    a_ring = nc.alloc_sbuf_tensor("a_ring", [128, WJ * IRING], f32)
    c_ring = nc.alloc_sbuf_tensor("c_ring", [128, WJ * IRING], bf16)
    r_ring = nc.alloc_sbuf_tensor("r_ring", [128, WJ * IRING], bf16)
    bias_t = nc.alloc_sbuf_tensor("bias_t", [128, 1], f32)
    vout_t = nc.alloc_sbuf_tensor("vout_t", [128, 1], bf16)

    PW = WPITCH * WRING   # w_ring partition pitch (elements)
    PI = WJ * IRING       # input ring partition pitch

    def wr_ap(p0, np_, foff, dims):
        return bass.AP(w_ring, p0 * PW + foff, [[PW, np_]] + dims)

    def ir_ap(ring, p0, np_, foff, dims):
        return bass.AP(ring, p0 * PI + foff, [[PI, np_]] + dims)

    def wslot(t):
        return (t % WRING) * WPITCH

    def islot(t):
        # input slot keyed to t-1 so G-windows [Gk+1, G(k+1)+1) never wrap
        return ((t - 1) % IRING) * WJ

    with tile.TileContext(nc) as tc, ExitStack() as ctx:
        tmp_pool = ctx.enter_context(tc.tile_pool(name="tmp", bufs=8))
        s_pool = ctx.enter_context(tc.tile_pool(name="sprep", bufs=3))

        # ---- one-time init ----
        nc.gpsimd.memset(bias_t.ap(), -al)
        nc.vector.memset(w_ring.ap(), 0.0)
        nc.gpsimd.memset(c_ring.ap(), 0.0)
        nc.gpsimd.memset(r_ring.ap(), 0.0)
        nc.gpsimd.memset(th_ring.ap(), 0.0)
        nc.gpsimd.memset(a_ring.ap(), 0.0)

        def emit_input_row(t):
            """Gather theta & A rows for step t: ONE DMA per tensor covering
            all active strips (partition dim spans 16*(hi-lo) partitions, so
            the cost model's per-partition-byte charge is minimal).  Strip s
            needs source row t-LAG*s-1; stride over s is WJ - LAG*M."""
            if t < 1 or t > T_TOTAL:
                return
            lo, hi = _active_range(t)
            if hi <= lo:
                return
            ns_ = hi - lo
            for j, (dram, ring) in enumerate(
                    ((th_d, th_ring), (a_d, a_ring))):
                eng = (nc.sync, nc.gpsimd, nc.sync, nc.scalar, nc.gpsimd,
                       nc.sync, nc.gpsimd, nc.sync, nc.scalar, nc.sync,
                       nc.gpsimd, nc.sync, nc.scalar, nc.gpsimd, nc.sync,
                       nc.gpsimd, nc.sync, nc.scalar, nc.gpsimd, nc.sync,
                       )[(2 * t + j) % 20]
                src = bass.AP(
                    dram,
                    (t - LAG * (hi - 1) - 1) * M + WJ * (hi - 1),
                    [[LAG * M - WJ, ns_], [N * M, B], [1, WJ]],
                )
                dst = bass.AP(ring, 16 * (S - hi) * PI + islot(t),
                              [[PI, 16 * ns_], [1, WJ]])
                eng.dma_start(dst, src)

        def emit_prep_window(t0):
            """s = theta + A (pool, f32 tile); c = exp(s - al) bf16;
            r = exp(-A - al) bf16."""
            t0 = max(t0, 1)
            tend = min(t0 + GP, T_TOTAL + 1)
            if t0 >= tend:
                return
            nt = tend - t0
            th_s = ir_ap(th_ring, 0, 128, islot(t0), [[1, WJ * nt]])
            a_s = ir_ap(a_ring, 0, 128, islot(t0), [[1, WJ * nt]])
            c_s = ir_ap(c_ring, 0, 128, islot(t0), [[1, WJ * nt]])
            r_s = ir_ap(r_ring, 0, 128, islot(t0), [[1, WJ * nt]])
            s_t = s_pool.tile([128, WJ * GP], f32, tag="s")
            nc.gpsimd.tensor_tensor(s_t[:, 0:WJ * nt], th_s, a_s, Alu.add)
            nc.scalar.activation(c_s, s_t[:, 0:WJ * nt], ActFn.Exp,
                                 bias=bias_t.ap())
            nc.scalar.activation(r_s, a_s, ActFn.Exp, bias=bias_t.ap(),
                                 scale=-1.0)

        def emit_guard_window(w0):
            """Handoff: dst slots [w0, w0+GW) col0 p16..128 <- src slots
            [w0-LAG, ..) col64 p0..112 (pool SWDGE queue)."""
            if w0 >= T_TOTAL:
                return
            gw = min(GW, T_TOTAL + 1 - w0)
            with nc.allow_non_contiguous_dma(reason="strip handoff scatter"):
                src = wr_ap(16, 112, wslot(w0 - LAG) + WJ, [[WPITCH, gw]])
                dst = wr_ap(0, 112, wslot(w0) + 0, [[WPITCH, gw]])
                nc.scalar.dma_start(dst, src)

        def emit_btab(ta, tb):
            """Strip-0 left boundary exp(-al*i) into col0 of slots [ta, tb),
            split at W-ring wrap boundaries."""
            ta = max(ta, 1)
            tb = min(tb, N + 1)
            while ta < tb:
                te = min(tb, (ta // WRING + 1) * WRING)  # stop at ring wrap
                with nc.allow_non_contiguous_dma(reason="boundary scatter"):
                    bsrc = bass.AP(bt_c, ta - 1, [[N, B], [1, te - ta]])
                    bdst = wr_ap(112, 16, wslot(ta) + 0, [[WPITCH, te - ta]])
                    nc.scalar.dma_start(bdst, bsrc)
                ta = te

        def emit_prefill(s):
            """Row-0 slot content for strip s into slot (LAG*s)."""
            p0 = 16 * (S - 1 - s)
            src = bass.AP(w0_c, p0 * WPITCH, [[WPITCH, 16], [1, WPITCH]])
            dst = wr_ap(p0, 16, wslot(LAG * s), [[1, WPITCH]])
            nc.scalar.dma_start(dst, src)

        # ---- prologue ----
        emit_prefill(0)
        for u in range(1, PF_IN + 1):
            emit_input_row(u)
        emit_btab(1, GB + GB // 2)
        for t0 in range(1, PF_PREP + 1, GP):
            emit_prep_window(t0)
        for w0 in range(0, PF_GUARD + GW, GW):
            emit_guard_window(w0)

        # ---- main unrolled loop ----
        for t in range(1, T_TOTAL + 1):
            emit_input_row(t + PF_IN)
            if t % GP == 1:
                emit_prep_window(t + PF_PREP)
            if t % GW == 0:
                emit_guard_window(t + PF_GUARD)
            if t % GB == 0:
                emit_btab(t + GB // 2, t + GB + GB // 2)

            wp = wslot(t - 1)        # previous row slot
            wc = wslot(t)            # current row slot
            ci = islot(t)
            m_t = tmp_pool.tile([128, WJ], bf16, tag="m")
            z_t = tmp_pool.tile([128, WJ], bf16, tag="z")
            # m = r * Wprev[j-1]
            nc.vector.tensor_tensor(
                m_t[:], ir_ap(r_ring, 0, 128, ci, [[1, WJ]]),
                wr_ap(0, 128, wp + 0, [[1, WJ]]), Alu.mult)
            # z = m + Wprev[j]
            nc.vector.tensor_tensor(
                z_t[:], m_t[:], wr_ap(0, 128, wp + 1, [[1, WJ]]), Alu.add)
            # W[:, j] = (z[j] + state) * c[j],  state0 = col0 boundary
            nc.vector.tensor_tensor_scan(
                wr_ap(0, 128, wc + 1, [[1, WJ]]),
                z_t[:],
                ir_ap(c_ring, 0, 128, ci, [[1, WJ]]),
                wr_ap(0, 128, wc + 0, [[1, 1]]),
                op0=Alu.add, op1=Alu.mult)

            if t % LAG == 0 and t // LAG < S:
                emit_prefill(t // LAG)

        # ---- finale: V = log(W[N, M]) + alpha*(N+M) ----
        fin = wslot(T_TOTAL) + WJ
        vlog_t = tmp_pool.tile([128, 1], f32, tag="vlog")
        nc.scalar.activation(vlog_t[0:B, 0:1], wr_ap(0, B, fin, [[1, 1]]),
                             ActFn.Ln)
        nc.vector.tensor_scalar_add(
            vlog_t[0:B, 0:1], vlog_t[0:B, 0:1], al * (N + M))
        nc.sync.dma_start(
            bass.AP(out_d, 0, [[1, B], [1, 1]]), vlog_t[0:B, 0:1])

    nc.finalize()
    return nc


def _get_nc():
    if "nc" not in _CACHE:
        _CACHE["nc"] = _build_nc()
    return _CACHE["nc"]


def kernel(theta, A):
    from concourse.bass_utils import run_bass_kernel_spmd

    theta = np.ascontiguousarray(np.asarray(theta, dtype=np.float32))
    A = np.ascontiguousarray(np.asarray(A, dtype=np.float32))
    nc = _get_nc()
    in_maps = [
        {"theta": theta[c * B:(c + 1) * B], "A": A[c * B:(c + 1) * B]}
        for c in range(NCORES)
    ]
    res = run_bass_kernel_spmd(nc, in_maps, core_ids=list(range(NCORES)))
    return np.concatenate([r["out"].reshape(B) for r in res.results]).astype(np.float32)


# revision 23
# speedup vs baseline: 1.0025x; 1.0025x over previous
"""Soft Needleman-Wunsch forward (logsumexp semiring) on Trainium2, 8 cores.

Exp-space linearization: W[i,j] = exp(V[i,j] - alpha*(i+j)) satisfies
    W[i,j] = (z[j] + W[i,j-1]) * c[i,j]
    z[j]   = W[i-1,j] + r[i,j] * W[i-1,j-1]
    c = exp(theta + A - alpha),  r = exp(-A - alpha)
so each DP row is one tensor_tensor_scan (op0=add, op1=mult) plus two
elementwise DVE ops (m = r*Wshift, z = m + Wprev).

Per core (16 batch): 8 column strips of 64, partition p = 16*s + b,
strip s lags s-1 by LAG rows (wavefront); row i of strip s runs at
step t = i + LAG*s.  Strip handoff (col64 -> next strip col0) is one
SBUF->SBUF DMA per GW steps.

Performance structure: c/r/W rings and the DP ops are bf16 (DVE 2x mode
on the tensor_tensor ops), LAG=16 (624 steps).  Strips map to partitions
REVERSED (strip s on partitions 16*(S-1-s)..) so that one input DMA per
step can gather theta/A rows for ALL active strips with positive strides
and a full-width (up to 128-partition) destination -- the DMA cost model
charges per-partition bytes, so wide-partition DMAs are ~8x cheaper than
per-strip ones.  Input DMAs round-robin over the SP/Pool/ACT queues.
"""
import numpy as np

# ---- constants (self-contained; hardcoded for the 128x512x512 problem) ----
B_FULL, N, M = 128, 512, 512
NCORES = 8
B = B_FULL // NCORES      # 16 batch elements per core
S = 8                     # column strips
WJ = M // S               # 64 = strip width
LAG = 16                  # inter-strip row lag (steps)
GW = 8                    # handoff batching window (steps); must be < LAG
T_TOTAL = N + (S - 1) * LAG   # 624 steps
WRING = 64                # W row ring depth (slots); multiple of GW and LAG
WPITCH = WJ + 1           # 65 elems per slot per partition
IRING = 128               # input ring depth (steps)
G = 64                    # input DMA window (steps); IRING/2 (aligned)
GP = 16                   # prep (sum/exp) grouping (steps)
PF_IN = 88                # input DMA issue lead (steps)
PF_PREP = 48              # exp/sum prep prefetch distance
PF_GUARD = 8              # handoff DMA prefetch distance
GB = 32                   # strip-0 boundary (btab) window (steps)
ALPHA = 1.52              # calibrated exp-space drift rate

_CACHE = {}


def _bf16(x):
    import ml_dtypes
    return np.asarray(x, dtype=ml_dtypes.bfloat16)


def _active_range(t):
    """Strips s with 1 <= t - LAG*s <= N, as [lo, hi) (contiguous range)."""
    lo = max(0, -(-(t - N) // LAG))      # ceil((t - N) / LAG)
    hi = min(S - 1, (t - 1) // LAG) + 1
    return (lo, hi) if hi > lo else (0, 0)


def _build_nc():
    import concourse.bass as bass
    import concourse.bacc as bacc
    import concourse.mybir as mybir
    import concourse.tile as tile
    from contextlib import ExitStack

    f32 = mybir.dt.float32
    bf16 = mybir.dt.bfloat16
    Alu = mybir.AluOpType
    ActFn = mybir.ActivationFunctionType
    al = float(ALPHA)

    nc = bacc.Bacc("TRN2", target_bir_lowering=False)
    th_d = nc.dram_tensor("theta", [B, N, M], f32, kind="ExternalInput")
    a_d = nc.dram_tensor("A", [B, N, M], f32, kind="ExternalInput")
    out_d = nc.dram_tensor("out", [B, 1], f32, kind="ExternalOutput")

    # baked constant tables (bf16, matching the ring dtypes)
    sv = (S - 1 - np.arange(128) // B).astype(np.float64)
    fv = np.arange(WPITCH, dtype=np.float64)
    w0_np = np.exp(-al * (WJ * sv[:, None] + fv[None, :]))
    w0_c = nc.inline_tensor(_bf16(w0_np), name="w0tab")     # [128, 65]
    bt_np = np.tile(np.exp(-al * np.arange(1, N + 1, dtype=np.float64)
                           )[None, :], (B, 1))
    bt_c = nc.inline_tensor(_bf16(bt_np), name="btab")      # [16, 512]

    # persistent SBUF rings
    w_ring = nc.alloc_sbuf_tensor("w_ring", [128, WPITCH * WRING], bf16)
    th_ring = nc.alloc_sbuf_tensor("th_ring", [128, WJ * IRING], f32)
    a_ring = nc.alloc_sbuf_tensor("a_ring", [128, WJ * IRING], f32)
    c_ring = nc.alloc_sbuf_tensor("c_ring", [128, WJ * IRING], bf16)
    r_ring = nc.alloc_sbuf_tensor("r_ring", [128, WJ * IRING], bf16)
    bias_t = nc.alloc_sbuf_tensor("bias_t", [128, 1], f32)
    vout_t = nc.alloc_sbuf_tensor("vout_t", [128, 1], bf16)

    PW = WPITCH * WRING   # w_ring partition pitch (elements)
    PI = WJ * IRING       # input ring partition pitch

    def wr_ap(p0, np_, foff, dims):
        return bass.AP(w_ring, p0 * PW + foff, [[PW, np_]] + dims)

    def ir_ap(ring, p0, np_, foff, dims):
        return bass.AP(ring, p0 * PI + foff, [[PI, np_]] + dims)

    def wslot(t):
        return (t % WRING) * WPITCH

    def islot(t):
        # input slot keyed to t-1 so G-windows [Gk+1, G(k+1)+1) never wrap
        return ((t - 1) % IRING) * WJ

    with tile.TileContext(nc) as tc, ExitStack() as ctx:
        tmp_pool = ctx.enter_context(tc.tile_pool(name="tmp", bufs=8))
        s_pool = ctx.enter_context(tc.tile_pool(name="sprep", bufs=3))

        # ---- one-time init ----
        nc.gpsimd.memset(bias_t.ap(), -al)
        nc.vector.memset(w_ring.ap(), 0.0)
        nc.gpsimd.memset(c_ring.ap(), 0.0)
        nc.gpsimd.memset(r_ring.ap(), 0.0)
        nc.gpsimd.memset(th_ring.ap(), 0.0)
        nc.gpsimd.memset(a_ring.ap(), 0.0)

        def emit_input_row(t):
            """Gather theta & A rows for step t: ONE DMA per tensor covering
            all active strips (partition dim spans 16*(hi-lo) partitions, so
            the cost model's per-partition-byte charge is minimal).  Strip s
            needs source row t-LAG*s-1; stride over s is WJ - LAG*M."""
            if t < 1 or t > T_TOTAL:
                return
            lo, hi = _active_range(t)
            if hi <= lo:
                return
            ns_ = hi - lo
            for j, (dram, ring) in enumerate(
                    ((th_d, th_ring), (a_d, a_ring))):
                eng = (nc.sync, nc.gpsimd, nc.sync, nc.scalar, nc.gpsimd,
                       nc.sync, nc.gpsimd, nc.sync, nc.scalar, nc.sync,
                       nc.gpsimd, nc.sync, nc.scalar, nc.gpsimd, nc.sync,
                       nc.gpsimd, nc.sync, nc.scalar, nc.gpsimd, nc.sync,
                       )[(2 * t + j) % 20]
                src = bass.AP(
                    dram,
                    (t - LAG * (hi - 1) - 1) * M + WJ * (hi - 1),
                    [[LAG * M - WJ, ns_], [N * M, B], [1, WJ]],
                )
                dst = bass.AP(ring, 16 * (S - hi) * PI + islot(t),
                              [[PI, 16 * ns_], [1, WJ]])
                eng.dma_start(dst, src)

        def emit_prep_window(t0):
            """s = theta + A (pool, f32 tile); c = exp(s - al) bf16;
            r = exp(-A - al) bf16."""
            t0 = max(t0, 1)
            tend = min(t0 + GP, T_TOTAL + 1)
            if t0 >= tend:
                return
            nt = tend - t0
            th_s = ir_ap(th_ring, 0, 128, islot(t0), [[1, WJ * nt]])
            a_s = ir_ap(a_ring, 0, 128, islot(t0), [[1, WJ * nt]])
            c_s = ir_ap(c_ring, 0, 128, islot(t0), [[1, WJ * nt]])
            r_s = ir_ap(r_ring, 0, 128, islot(t0), [[1, WJ * nt]])
            s_t = s_pool.tile([128, WJ * GP], f32, tag="s")
            nc.gpsimd.tensor_tensor(s_t[:, 0:WJ * nt], th_s, a_s, Alu.add)
            nc.scalar.activation(c_s, s_t[:, 0:WJ * nt], ActFn.Exp,
                                 bias=bias_t.ap())
            nc.scalar.activation(r_s, a_s, ActFn.Exp, bias=bias_t.ap(),
                                 scale=-1.0)

        def emit_guard_window(w0):
            """Handoff: dst slots [w0, w0+GW) col0 p16..128 <- src slots
            [w0-LAG, ..) col64 p0..112 (pool SWDGE queue)."""
            if w0 >= T_TOTAL:
                return
            gw = min(GW, T_TOTAL + 1 - w0)
            with nc.allow_non_contiguous_dma(reason="strip handoff scatter"):
                src = wr_ap(16, 112, wslot(w0 - LAG) + WJ, [[WPITCH, gw]])
                dst = wr_ap(0, 112, wslot(w0) + 0, [[WPITCH, gw]])
                nc.scalar.dma_start(dst, src)

        def emit_btab(ta, tb):
            """Strip-0 left boundary exp(-al*i) into col0 of slots [ta, tb),
            split at W-ring wrap boundaries."""
            ta = max(ta, 1)
            tb = min(tb, N + 1)
            while ta < tb:
                te = min(tb, (ta // WRING + 1) * WRING)  # stop at ring wrap
                with nc.allow_non_contiguous_dma(reason="boundary scatter"):
                    bsrc = bass.AP(bt_c, ta - 1, [[N, B], [1, te - ta]])
                    bdst = wr_ap(112, 16, wslot(ta) + 0, [[WPITCH, te - ta]])
                    nc.scalar.dma_start(bdst, bsrc)
                ta = te

        def emit_prefill(s):
            """Row-0 slot content for strip s into slot (LAG*s)."""
            p0 = 16 * (S - 1 - s)
            src = bass.AP(w0_c, p0 * WPITCH, [[WPITCH, 16], [1, WPITCH]])
            dst = wr_ap(p0, 16, wslot(LAG * s), [[1, WPITCH]])
            nc.scalar.dma_start(dst, src)

        # ---- prologue ----
        emit_prefill(0)
        for u in range(1, PF_IN + 1):
            emit_input_row(u)
        emit_btab(1, GB + GB // 2)
        for t0 in range(1, PF_PREP + 1, GP):
            emit_prep_window(t0)
        for w0 in range(0, PF_GUARD + GW, GW):
            emit_guard_window(w0)

        # ---- main unrolled loop ----
        for t in range(1, T_TOTAL + 1):
            emit_input_row(t + PF_IN)
            if t % GP == 1:
                emit_prep_window(t + PF_PREP)
            if t % GW == 0:
                emit_guard_window(t + PF_GUARD)
            if t % GB == 0:
                emit_btab(t + GB // 2, t + GB + GB // 2)

            wp = wslot(t - 1)        # previous row slot
            wc = wslot(t)            # current row slot
            ci = islot(t)
            m_t = tmp_pool.tile([128, WJ], bf16, tag="m")
            z_t = tmp_pool.tile([128, WJ], bf16, tag="z")
            # m = r * Wprev[j-1]
            nc.vector.tensor_tensor(
                m_t[:], ir_ap(r_ring, 0, 128, ci, [[1, WJ]]),
                wr_ap(0, 128, wp + 0, [[1, WJ]]), Alu.mult)
            # z = m + Wprev[j]
            nc.vector.tensor_tensor(
                z_t[:], m_t[:], wr_ap(0, 128, wp + 1, [[1, WJ]]), Alu.add)
            # W[:, j] = (z[j] + state) * c[j],  state0 = col0 boundary
            nc.vector.tensor_tensor_scan(
                wr_ap(0, 128, wc + 1, [[1, WJ]]),
                z_t[:],
                ir_ap(c_ring, 0, 128, ci, [[1, WJ]]),
                wr_ap(0, 128, wc + 0, [[1, 1]]),
                op0=Alu.add, op1=Alu.mult)

            if t % LAG == 0 and t // LAG < S:
                emit_prefill(t // LAG)

        # ---- finale: V = log(W[N, M]) + alpha*(N+M) ----
        fin = wslot(T_TOTAL) + WJ
        vlog_t = tmp_pool.tile([128, 1], f32, tag="vlog")
        nc.scalar.activation(vlog_t[0:B, 0:1], wr_ap(0, B, fin, [[1, 1]]),
                             ActFn.Ln)
        nc.vector.tensor_scalar_add(
            vlog_t[0:B, 0:1], vlog_t[0:B, 0:1], al * (N + M))
        nc.sync.dma_start(
            bass.AP(out_d, 0, [[1, B], [1, 1]]), vlog_t[0:B, 0:1])

    nc.finalize()
    return nc


def _get_nc():
    if "nc" not in _CACHE:
        _CACHE["nc"] = _build_nc()
    return _CACHE["nc"]


def kernel(theta, A):
    from concourse.bass_utils import run_bass_kernel_spmd

    theta = np.ascontiguousarray(np.asarray(theta, dtype=np.float32))
    A = np.ascontiguousarray(np.asarray(A, dtype=np.float32))
    nc = _get_nc()
    in_maps = [
        {"theta": theta[c * B:(c + 1) * B], "A": A[c * B:(c + 1) * B]}
        for c in range(NCORES)
    ]
    res = run_bass_kernel_spmd(nc, in_maps, core_ids=list(range(NCORES)))
    return np.concatenate([r["out"].reshape(B) for r in res.results]).astype(np.float32)
